# revision 1
# baseline (speedup 1.0000x reference)
"""Trainium2 Bass kernel for nn_MaxPoolingMatching.

Reference computation (per batch b):
    v1[l,p,:] = l2norm(s1[l,:] * k[p,:])        # over d
    v2[m,p,:] = l2norm(s2[m,:] * k[p,:])
    cos[l,m,p] = <v1[l,p,:], v2[m,p,:]>
    out[l,p]   = max_m cos[l,m,p]

Rewritten as
    Y[l,m,p]  = sum_d s1[l,d] * k2[p,d] * s2[m,d]        (k2 = k*k)
    out[l,p]  = rinv1[l,p] * max_m ( Y[l,m,p] * rinv2[m,p] )
where rinv{1,2} are inverse weighted norms; the positive rinv1 factor
commutes with the max.

Mapping to the NeuronCore:
  - batch-parallel across 8 cores (2 batches per core), kernel replicated
  - main matmuls in float32r (full PE rate at N=256):
        lhsT = s1T * k2[p]  (k2 applied on ScalarE with per-partition scale)
        rhs  = s2T
  - rinv2 multiply fused into the max via DVE tensor_tensor_reduce, with
    rinv2 broadcast across partitions by an SBUF->SBUF DMA
  - weighted-norm sums via small PE matmuls on squared transposed inputs
  - transposes via PE (identity), squares on GPSIMD, copies/scales on ScalarE
"""

import sys

import numpy as np

if "/opt/trn_rl_repo" not in sys.path:
    sys.path.insert(0, "/opt/trn_rl_repo")

B, L, D, P = 16, 256, 256, 20
NCORES = 8
BLOC = B // NCORES  # batches per core
LC = L // 128  # l chunks (partition-dim tiles)
PSLAB = 4  # perspectives per PSUM slab
DC = D // 128  # d chunks (contraction tiles)

_NC_CACHE = {}

# Fused multiply + max-reduce as a custom DVE op (one pass over the matmul
# output instead of tensor_mul + tensor_reduce). Falls back to the two-pass
# epilogue when disabled.
USE_CUSTOM_DVE = True


def _register_mulmax_op():
    from concourse import dve_ops
    from concourse.dve_spec import Spec, Src0, Src1, AluOp, lower
    from concourse.dve_spec import _has_src1 as has_src1
    from concourse.dve_ops import DveOpSpec

    for op in dve_ops.OPS:
        if op.name == "MULMAX_ANT":
            return op
    import numpy as _np

    def _ref(in0, in1, c0, c1, c2):
        out = in0 * in1
        return out, out.max(axis=-1, keepdims=True)

    spec = Spec(body=Src0 * Src1, accum=AluOp.MAX, reference=_ref)
    op = dve_ops.DveOp("MULMAX_ANT", spec, subdim=False, uops_sha={})
    dve_ops.OPS.append(op)
    dve_ops._SUB_OPCODE_FOR_NAME[op.name] = (
        dve_ops._CUSTOM_DVE_ROW_BASE + len(dve_ops.OPS) - 1
    )
    dve_ops.CUSTOM_DVE_SPECS[op.name] = spec
    for ver in ("v3", "v4"):
        try:
            s = DveOpSpec(
                name=op.name,
                opcode=dve_ops.get_dve_sub_opcode(op.name),
                uops=lower(spec, ver=ver),
                rd1_en=has_src1(spec),
            )
            op.uops_sha[ver] = s.sha(ver)
        except Exception:
            pass
    return op


def build_nc(loop_R=None):
    """Build the kernel module. loop_R wraps the body in a For_i repetition
    loop (benchmarking only)."""
    import concourse.bass as bass
    import concourse.bacc as bacc
    import concourse.tile as tile
    from concourse import mybir
    from concourse.masks import make_identity
    from contextlib import ExitStack

    f32 = mybir.dt.float32
    f32r = mybir.dt.float32r
    Alu = mybir.AluOpType
    Act = mybir.ActivationFunctionType

    mulmax_op = _register_mulmax_op() if USE_CUSTOM_DVE else None

    nc = bacc.Bacc("TRN2", target_bir_lowering=False, debug=False)
    s1_d = nc.dram_tensor("sent1", [BLOC, L, D], f32, kind="ExternalInput").ap()
    s2_d = nc.dram_tensor("sent2", [BLOC, L, D], f32, kind="ExternalInput").ap()
    kr_d = nc.dram_tensor("kernel", [P, D], f32, kind="ExternalInput").ap()
    out_d = nc.dram_tensor("out", [BLOC, L, P], f32, kind="ExternalOutput").ap()

    with ExitStack() as ctx:
        tc = ctx.enter_context(tile.TileContext(nc))
        consts = ctx.enter_context(tc.tile_pool(name="consts", bufs=1))
        nat = ctx.enter_context(tc.tile_pool(name="nat", bufs=4))
        big = ctx.enter_context(tc.tile_pool(name="big", bufs=1))
        small = ctx.enter_context(tc.tile_pool(name="small", bufs=4))
        r2bp = ctx.enter_context(tc.tile_pool(name="r2bp", bufs=2))
        scrp = ctx.enter_context(tc.tile_pool(name="scrp", bufs=2))
        outp = ctx.enter_context(tc.tile_pool(name="outp", bufs=4))
        dramp = ctx.enter_context(tc.tile_pool(name="dram", bufs=2, space="DRAM"))
        ps_misc = ctx.enter_context(tc.tile_pool(name="ps_misc", bufs=3, space="PSUM"))
        ps_z = ctx.enter_context(tc.tile_pool(name="ps_z", bufs=2, space="PSUM"))

        # ---- constants -------------------------------------------------
        ident = consts.tile([128, 128], f32, tag="ident")
        make_identity(nc, ident)

        def emit_body():
            kr = consts.tile([P, D], f32, tag="kr")
            nc.sync.dma_start(out=kr, in_=kr_d)
            k2 = consts.tile([P, D], f32, tag="k2")
            nc.gpsimd.tensor_mul(k2, kr, kr)

            # k2T[d, p] per d-chunk: transpose k2 through the PE
            k2T = consts.tile([128, DC, P], f32, tag="k2T")
            for dc in range(DC):
                pst = ps_misc.tile([128, 256], f32, tag="misc")
                nc.tensor.transpose(
                    pst[:, :P], k2[:, dc * 128 : (dc + 1) * 128], ident[:P, :P]
                )
                nc.scalar.copy(out=k2T[:, dc, :], in_=pst[:, :P])

            # ---- load + transpose inputs ----------------------------------
            # s{1,2}T layout: [128(d), dc, b, l]  (b batches side by side)
            # s2T is written as float32r (pre-rounded) since it feeds the f32r
            # main matmuls; bitcast back to f32 for non-matmul readers.
            s1T = big.tile([128, DC, BLOC, L], f32, tag="s1T")
            s2T = big.tile([128, DC, BLOC, L], f32r, tag="s2T")
            for b in range(BLOC):
                for lc in range(LC):
                    for src, dst in ((s1_d, s1T), (s2_d, s2T)):
                        natt = nat.tile([128, D], f32, tag="nat")
                        nc.sync.dma_start(
                            out=natt, in_=src[b, lc * 128 : (lc + 1) * 128, :]
                        )
                        for dc in range(DC):
                            pst = ps_misc.tile([128, 256], f32, tag="misc")
                            nc.tensor.transpose(
                                pst[:, :128], natt[:, dc * 128 : (dc + 1) * 128], ident
                            )
                            nc.scalar.copy(
                                out=dst[:, dc, b, lc * 128 : (lc + 1) * 128],
                                in_=pst[:, :128],
                            )

            # squares (for the weighted norms) on GPSIMD
            s1Tsq = big.tile([128, DC, BLOC, L], f32, tag="s1Tsq")
            s2Tsq = big.tile([128, DC, BLOC, L], f32, tag="s2Tsq")
            s2Tf = s2T.bitcast(f32)
            for dc in range(DC):
                nc.gpsimd.tensor_mul(s1Tsq[:, dc], s1T[:, dc], s1T[:, dc])
                nc.gpsimd.tensor_mul(s2Tsq[:, dc], s2Tf[:, dc], s2Tf[:, dc])

            # ---- weighted norms -> rinv1 [l,p], rinv2T [p,m] ---------------
            eps_t = consts.tile([128, 1], f32, tag="eps")
            nc.vector.memset(eps_t, 1e-12)
            rinv1 = {}
            for b in range(BLOC):
                for lc in range(LC):
                    psn = ps_misc.tile([128, 256], f32, tag="misc")
                    for dc in range(DC):
                        nc.tensor.matmul(
                            psn[:, :P],
                            s1Tsq[:, dc, b, lc * 128 : (lc + 1) * 128],
                            k2T[:, dc, :],
                            start=(dc == 0),
                            stop=(dc == DC - 1),
                        )
                    sq1 = small.tile([128, P], f32, tag="sq1")
                    nc.scalar.activation(
                        out=sq1, in_=psn[:, :P], func=Act.Sqrt, bias=eps_t, scale=1.0
                    )
                    r1 = small.tile([128, P], f32, tag="rinv1")
                    nc.vector.reciprocal(out=r1, in_=sq1)
                    rinv1[b, lc] = r1

            rinv2T = {}
            for b in range(BLOC):
                psn = ps_misc.tile([128, 256], f32, tag="misc")
                for dc in range(DC):
                    nc.tensor.matmul(
                        psn[:P, :],
                        k2T[:, dc, :],
                        s2Tsq[:, dc, b, :],
                        start=(dc == 0),
                        stop=(dc == DC - 1),
                    )
                sq2 = small.tile([P, L], f32, tag="sq2")
                nc.scalar.activation(
                    out=sq2, in_=psn[:P, :], func=Act.Sqrt, bias=eps_t[:P], scale=1.0
                )
                r2 = small.tile([P, L], f32, tag="rinv2T")
                nc.vector.reciprocal(out=r2, in_=sq2)
                rinv2T[b] = r2

            # ---- k2-scaled stationary operand (written as float32r) --------
            # lhsK[d, dc, p, b, l] = s1T[d, dc, b, l] * k2T[d, dc, p]
            lhsK = big.tile([128, DC, P, BLOC, L], f32r, tag="lhsK")
            for dc in range(DC):
                for p in range(P):
                    nc.scalar.activation(
                        out=lhsK[:, dc, p],
                        in_=s1T[:, dc],
                        func=Act.Copy,
                        scale=k2T[:, dc, p : p + 1],
                    )

            # ---- main loop --------------------------------------------------
            for b in range(BLOC):
                # rinv2 broadcast tile [128, p, m]: bounce through DRAM, then
                # partition-broadcast DMA reads (step-0 partition APs are legal
                # for DRAM sources)
                r2d = dramp.tile([P, L], f32, tag="r2d")
                nc.sync.dma_start(out=r2d, in_=rinv2T[b])
                r2b = r2bp.tile([128, P, L], f32, tag="r2b")
                r2d_flat = r2d.rearrange("a b -> (a b)")
                # broadcast in p-slab chunks on alternating HWDGE queues so the
                # first epilogue slab isn't gated on the full 2.5MB transfer
                for ci, ps in enumerate(range(0, P, PSLAB)):
                    nps = min(PSLAB, P - ps)
                    chunk = r2d_flat[ps * L : (ps + nps) * L]
                    chunk_bcast = bass.AP(
                        tensor=chunk.tensor,
                        offset=chunk.offset,
                        ap=[[0, 128]] + list(chunk.ap),
                    )
                    eng = nc.sync if ci % 2 == 0 else nc.scalar
                    eng.dma_start(out=r2b[:, ps : ps + nps, :], in_=chunk_bcast)

                for lc in range(LC):
                    maxt = outp.tile([128, P], f32, tag="maxt")
                    for ps in range(0, P, PSLAB):
                        nps = min(PSLAB, P - ps)
                        psz = ps_z.tile([128, PSLAB, L], f32, tag="z")
                        for i in range(nps):
                            for dc in range(DC):
                                nc.tensor.matmul(
                                    psz[:, i, :],
                                    lhsK[:, dc, ps + i, b, lc * 128 : (lc + 1) * 128],
                                    s2T[:, dc, b, :],
                                    start=(dc == 0),
                                    stop=(dc == DC - 1),
                                )
                        # scale by rinv2 (broadcast tile), then max over m
                        if USE_CUSTOM_DVE:
                            for i in range(nps):
                                scr = scrp.tile([128, L], f32, tag="scr")
                                nc.vector._custom_dve(
                                    mulmax_op,
                                    out=scr,
                                    in0=psz[:, i, :],
                                    in1=r2b[:, ps + i, :],
                                    accum_out=maxt[:, ps + i : ps + i + 1],
                                )
                        else:
                            scr2 = scrp.tile([128, PSLAB, L], f32, tag="scr2")
                            nc.vector.tensor_mul(
                                scr2[:, :nps, :], psz[:, :nps, :], r2b[:, ps : ps + nps, :]
                            )
                            nc.vector.tensor_reduce(
                                out=maxt[:, ps : ps + nps],
                                in_=scr2[:, :nps, :],
                                axis=mybir.AxisListType.X,
                                op=Alu.max,
                            )
                    outt = outp.tile([128, P], f32, tag="outt")
                    nc.vector.tensor_mul(outt, maxt, rinv1[b, lc])
                    nc.sync.dma_start(
                        out=out_d[b, lc * 128 : (lc + 1) * 128, :], in_=outt
                    )

        if loop_R is None:
            emit_body()
        else:
            with tc.For_i(0, loop_R, 1):
                emit_body()

    nc.compile()
    return nc


def _get_nc():
    if "nc" not in _NC_CACHE:
        _NC_CACHE["nc"] = build_nc()
    return _NC_CACHE["nc"]


def run(inputs, trace=False, trace_kwargs=None):
    from concourse.bass_utils import run_bass_kernel_spmd

    nc = _get_nc()
    sent1 = np.ascontiguousarray(np.asarray(inputs["sent1"], dtype=np.float32))
    sent2 = np.ascontiguousarray(np.asarray(inputs["sent2"], dtype=np.float32))
    kr = np.ascontiguousarray(np.asarray(inputs["kernel"], dtype=np.float32))
    in_maps = [
        {
            "sent1": sent1[i * BLOC : (i + 1) * BLOC],
            "sent2": sent2[i * BLOC : (i + 1) * BLOC],
            "kernel": kr,
        }
        for i in range(NCORES)
    ]
    res = run_bass_kernel_spmd(
        nc,
        in_maps,
        core_ids=list(range(NCORES)),
        trace=trace,
        **(trace_kwargs or {}),
    )
    out = np.concatenate([res.results[i]["out"] for i in range(NCORES)], axis=0)
    return out, res


def kernel(sent1, sent2, kernel):
    out, _ = run({"sent1": sent1, "sent2": sent2, "kernel": kernel})
    return out

